# revision 19
# baseline (speedup 1.0000x reference)
"""Trainium2 Bass kernel for causal multi-head attention (B=4, S=2048, E=1024, H=16).

Sharding: 8 cores = (batch b in 0..3) x (head-group g in 0..1); each core
computes one batch and 8 heads end-to-end:
  - column-parallel QKV projection (only its heads' columns)
  - causal attention for its 8 heads
  - row-parallel output projection -> partial [S, E]
The two partials per batch are summed on the host (plus b_proj). No on-device
collectives are needed.

Device dataflow (per core), all matmuls in bf16 with fp32 PSUM accumulation:
  - Q^T, K^T computed directly in [feature, token] layout (out = W^T @ x^T),
    so attention needs no transposes. K^T is pre-scaled by 1/sqrt(d)=0.125.
  - scores S^T[k,q] = (K^T tile).T @ Q^T, two k-tiles fused per [128,1024]
    PSUM tile; one ACT exp evacuates both; causal mask via multiplicative
    0/1 bf16 masks on diagonal tiles (fully-masked tiles skipped).
  - AV: lhsT = [V | ones]; PSUM rows 0..63 = O^T (unnormalized), row 64 =
    softmax denominator. 1/denom via ACT exp(-ln(d)) (DVE reciprocal is
    ~3.3us for a 1-partition row; ACT is ~0.9us), PE ones-matmul broadcast,
    one DVE multiply -> normalized O^T bf16. Normalize chains are deferred
    into the next (head, q-block) group so the PE never stalls on them.
  - All PSUM evacuation goes through ACT: DVE reads of PE-written PSUM race
    the matmul drain on HW (observed flaky garbage on first execution).
  - c_proj contracts the stacked O^T [512, S] against W_proj rows.
"""

import os
import sys

import numpy as np

for _p in ("/opt/trn_rl_repo", "/root/.axon_site/_ro/trn_rl_repo"):
    if os.path.isdir(_p) and _p not in sys.path:
        sys.path.append(_p)

import ml_dtypes  # noqa: E402

import concourse.bass as bass  # noqa: E402
import concourse.tile as tile  # noqa: E402
from concourse import bacc, bass_utils, hw_specs, mybir  # noqa: E402

# The act-table chooser assigns each activation the first table set containing
# its function, which ping-pongs Exp ("exp_and_others") and Ln ("natural_log")
# and inserts a 1.3us ACT_TABLE_LOAD per normalize chain (~65 per program).
# Restrict Exp/Ln to the combined "natural_log_exp_and_others" set (its
# act_func_set_id is preserved) so one load covers the whole kernel.
_orig_gat = hw_specs.get_activation_tables


def _shaped_gat(arch):
    t = _orig_gat(arch)
    if "natural_log_exp_and_others" in t:
        for name, funcs in t.items():
            if name != "natural_log_exp_and_others":
                funcs.discard(mybir.ActivationFunctionType.Exp)
                funcs.discard(mybir.ActivationFunctionType.Ln)
    return t


hw_specs.get_activation_tables = _shaped_gat
bacc.get_activation_tables = _shaped_gat

BF16 = ml_dtypes.bfloat16

B, S, E, H = 4, 2048, 1024, 16
D = E // H            # 64
NCORE = 8
HPC = H // 2          # heads per core = 8
KT = E // 128         # contraction tiles over E = 8
TB512 = S // 512      # 4
TB128 = S // 128      # 16
FPC = HPC * D         # features per core in attention output = 512

_cache: dict = {}


def _ts(i, n):
    return slice(i * n, (i + 1) * n)


def _build_program(with_bias):
    bf = mybir.dt.bfloat16
    f32 = mybir.dt.float32
    nc = bacc.Bacc("TRN2", target_bir_lowering=False, debug=False)

    xt = nc.dram_tensor("xt", [128, KT, S], bf, kind="ExternalInput")
    wqk = nc.dram_tensor("wqk", [128, KT, 2 * FPC], bf, kind="ExternalInput")
    wv = nc.dram_tensor("wv", [128, KT, FPC], bf, kind="ExternalInput")
    wp = nc.dram_tensor("wp", [128, FPC // 128, E], bf, kind="ExternalInput")
    if with_bias:
        bqk = nc.dram_tensor("bqk", [1, 2 * FPC], bf, kind="ExternalInput")
        bv = nc.dram_tensor("bv", [1, FPC], bf, kind="ExternalInput")
    msk = nc.dram_tensor("msk", [128, 4, 512], bf, kind="ExternalInput")
    out = nc.dram_tensor("out", [S, E], f32, kind="ExternalOutput")
    # scratch for the softmax-reciprocal partition broadcast (DMA bounce)
    rscr = nc.dram_tensor("rscr", [HPC * TB512, 512], mybir.dt.float32)

    Exp = mybir.ActivationFunctionType.Exp
    Ln = mybir.ActivationFunctionType.Ln

    with tile.TileContext(nc) as tc:
        with (
            tc.tile_pool(name="big", bufs=1) as big,
            tc.tile_pool(name="pp", bufs=3) as pp,
            tc.tile_pool(name="sm", bufs=3) as sm,
            tc.tile_pool(name="ob", bufs=3) as ob,
        ):
            xts = [
                big.tile([128, S], bf, tag=f"xt{k}", name=f"xt{k}")
                for k in range(KT)
            ]
            wqks = [
                big.tile([128, 2 * FPC], bf, tag=f"wqk{k}", name=f"wqk{k}")
                for k in range(KT)
            ]
            wv_sb = big.tile([128, KT, FPC], bf, tag="wv")
            wp_sb = big.tile([128, FPC // 128, E], bf, tag="wp")
            if with_bias:
                bqk_sb = big.tile([1, 2 * FPC], bf, tag="bqk")
                bv_sb = big.tile([1, FPC], bf, tag="bv")
            msk_sb = big.tile([128, 4, 512], bf, tag="msk")
            ones_sb = big.tile([1, S], bf, tag="ones")
            # per-pair Q^T/K^T/O^T tiles so interleaved QKV writes don't
            # WAR-serialize against another pair's attention reads
            qTs = [big.tile([128, S], bf, tag=f"qT{p}", name=f"qT{p}") for p in range(4)]
            kTs = [big.tile([128, S], bf, tag=f"kT{p}", name=f"kT{p}") for p in range(4)]
            oTs = [big.tile([128, S], bf, tag=f"oT{p}", name=f"oT{p}") for p in range(4)]
            vone_k = [
                big.tile([128, 2, HPC, D + 1], bf, tag=f"vone{tp}", name=f"vone{tp}")
                for tp in range(TB128 // 2)
            ]

            if with_bias:
                nc.sync.dma_start(out=bqk_sb, in_=bqk.ap())
                nc.sync.dma_start(out=bv_sb, in_=bv.ap())
            for k in range(KT):
                nc.sync.dma_start(out=wqks[k], in_=wqk.ap()[:, k, :])
                nc.sync.dma_start(out=xts[k], in_=xt.ap()[:, k, :])
            nc.sync.dma_start(out=wv_sb, in_=wv.ap())
            nc.sync.dma_start(out=msk_sb, in_=msk.ap())
            nc.sync.dma_start(out=wp_sb, in_=wp.ap())

            nc.vector.memset(ones_sb, 1.0)
            for tp in range(TB128 // 2):
                nc.vector.memset(vone_k[tp][:, :, :, D : D + 1], 1.0)

            # Attention-phase PSUM pools are also used to double-buffer the
            # upfront QKV/V projection groups (same [128,1024] tile shape).
            with (
                tc.tile_pool(name="sp", bufs=2, space="PSUM") as ps_sp,
                tc.tile_pool(name="av", bufs=2, space="PSUM") as ps_av,
                tc.tile_pool(name="qk", bufs=1, space="PSUM") as ps_qk,
            ):

                def emit_qk_group(fb, tbp, pool, tag):
                    """One Q^T/K^T projection group: 18 matmuls + 1 cast.
                    Yields after each instruction so it can be interleaved."""
                    ps = pool.tile([128, 1024], f32, tag=tag, name=f"qkv_{fb}_{tbp}")
                    for half in range(2):
                        tb = 2 * tbp + half
                        hs = _ts(half, 512)
                        for kt in range(KT):
                            nc.tensor.matmul(
                                ps[:, hs],
                                lhsT=wqks[kt][:, _ts(fb, 128)],
                                rhs=xts[kt][:, _ts(tb, 512)],
                                start=(kt == 0),
                                stop=(kt == KT - 1 and not with_bias),
                            )
                            yield
                        if with_bias:
                            nc.tensor.matmul(
                                ps[:, hs],
                                lhsT=bqk_sb[0:1, _ts(fb, 128)],
                                rhs=ones_sb[0:1, _ts(tb, 512)],
                                start=False,
                                stop=True,
                            )
                            yield
                    if fb < 4:
                        nc.scalar.copy(qTs[fb][:, _ts(tbp, 1024)], ps)
                    else:
                        nc.scalar.mul(kTs[fb - 4][:, _ts(tbp, 1024)], ps, 0.125)
                    yield

                # ---- Phase 1a: pair-0 Q^T/K^T upfront (dense) ----
                pools = [ps_qk, ps_sp]
                gi = 0
                for fb in (0, 4):
                    for tbp in range(TB512 // 2):
                        pool = pools[gi % 2]
                        for _ in emit_qk_group(
                            fb, tbp, pool, "qkvi" if pool is ps_qk else "sp"
                        ):
                            pass
                        gi += 1

                # ---- Phase 1b: V = x @ Wv (+bias), [token, feature] ----
                def emit_v_group(tbp, pool, tag):
                    ps = pool.tile([128, 1024], f32, tag=tag, name=f"v_{tbp}")
                    for half in range(2):
                        tb = 2 * tbp + half
                        hs = _ts(half, 512)
                        for kt in range(KT):
                            nc.tensor.matmul(
                                ps[:, hs],
                                lhsT=xts[kt][:, _ts(tb, 128)],
                                rhs=wv_sb[:, kt, :],
                                start=(kt == 0),
                                stop=(kt == KT - 1 and not with_bias),
                            )
                            yield
                        if with_bias:
                            nc.tensor.matmul(
                                ps[:, hs],
                                lhsT=ones_sb[0:1, 0:128],
                                rhs=bv_sb[0:1, :],
                                start=False,
                                stop=True,
                            )
                            yield
                    nc.scalar.copy(
                        vone_k[tbp][:, :, :, 0:D],
                        ps[:, :].rearrange("p (t h d) -> p t h d", t=2, d=D),
                    )
                    yield

                # V tile-pairs 0..1 upfront (needed by the very first AVs)
                for tbp in range(2):
                    pool = pools[tbp % 2]
                    for _ in emit_v_group(
                        tbp, pool, "qkvi" if pool is ps_qk else "sp"
                    ):
                        pass

                # ---- feed: V tile-pairs 2..7 then QKV pairs 1..3,
                # interleaved into attention to keep the PE dense ----
                def main_feed():
                    for tbp in range(2, TB128 // 2):
                        yield from (
                            ("v", tbp) for _ in emit_v_group(tbp, ps_qk, "qkvi")
                        )
                    for pr in range(1, 4):
                        for fb in (pr, pr + 4):
                            for tbp in range(TB512 // 2):
                                yield from (
                                    ("qk", pr)
                                    for _ in emit_qk_group(fb, tbp, ps_qk, "qkvi")
                                )

                feed = main_feed()
                # cur_pr semantics: pairs < cur_pr are fully emitted
                feed_state = {"cur_v": 2, "cur_pr": 1, "alive": True}

                def pump(n):
                    for _ in range(n):
                        got = next(feed, None)
                        if got is None:
                            feed_state["alive"] = False
                            feed_state["cur_v"] = 99
                            feed_state["cur_pr"] = 5
                            return
                        kind, idx = got
                        if kind == "v":
                            feed_state["cur_v"] = idx
                        else:
                            feed_state["cur_v"] = 99
                            feed_state["cur_pr"] = idx

                def drain_feed_through(pr):
                    """Emit QKV work until every pair <= pr is complete.
                    Required before attention reads qTs[pr]/kTs[pr]: Tile only
                    sees RAW deps for writes emitted before the read."""
                    while feed_state["alive"] and feed_state["cur_pr"] <= pr:
                        pump(1)

                def drain_v_through(tp):
                    """Emit V work until vone tile-pairs <= tp are complete."""
                    while feed_state["alive"] and feed_state["cur_v"] <= tp:
                        pump(1)

                # ---- c_proj feed: interleaved into pair-3 attention as the
                # QKV feed runs dry there (its work must precede pair 3).
                # A tb group unlocks once every head has normalized its
                # q-block (pair 3 / h=7 is the last writer of oTs[3]).
                def cproj_group(tb):
                    ps = ps_qk.tile([128, 1024], f32, tag="qkvi", name=f"pj{tb}")
                    for eb in range(2):
                        for fg in range(FPC // 128):
                            nc.tensor.matmul(
                                ps[:, _ts(eb, 512)],
                                lhsT=oTs[fg][:, _ts(tb, 128)],
                                rhs=wp_sb[:, fg, _ts(eb, 512)],
                                start=(fg == 0),
                                stop=(fg == FPC // 128 - 1),
                            )
                            yield
                    o_sb = ob.tile([128, 1024], f32, tag="osb", name=f"ob{tb}")
                    nc.scalar.copy(o_sb, ps)
                    yield
                    nc.sync.dma_start(out=out.ap()[_ts(tb, 128), :], in_=o_sb)
                    yield

                def cproj_feed():
                    for tb in range(TB128):
                        while tb >= cp_state["unlocked"]:
                            yield False  # not allowed yet; no emission
                        yield from (True for _ in cproj_group(tb))

                cp_state = {"unlocked": 0}
                cfeed = cproj_feed()

                def pump_cproj(n):
                    for _ in range(n):
                        got = next(cfeed, None)
                        if got is None or got is False:
                            return

                # ---- Phase 2: causal attention per head ----
                deferred = []  # pending normalize chains (one per group)

                def normalize(av, pair, qoff, qb, idx):
                    t1 = sm.tile([1, 512], f32, tag="t1")
                    nc.scalar.activation(t1, av[D : D + 1, :], Ln)
                    r_sb = sm.tile([1, 512], f32, tag="r")
                    nc.scalar.activation(r_sb, t1, Exp, scale=-1.0)
                    # broadcast 1/denom across partitions via a DRAM bounce
                    nc.sync.dma_start(out=rscr.ap()[idx : idx + 1, :], in_=r_sb)
                    bc_sb = sm.tile([D, 512], f32, tag="bcsb")
                    rap = rscr.ap()[idx : idx + 1, :]
                    bcast = bass.AP(
                        tensor=rap.tensor,
                        offset=rap.offset,
                        ap=[[0, D]] + list(rap.ap)[1:],
                    )
                    nc.sync.dma_start(out=bc_sb, in_=bcast)
                    # av is read by DVE well after the AV matmuls drained
                    # (the chain is deferred a full group) so the direct
                    # PSUM read is safe here.
                    nc.vector.tensor_mul(
                        oTs[pair][qoff : qoff + D, _ts(qb, 512)], av[0:D, :], bc_sb
                    )

                step_no = [0]
                for h in range(HPC):
                    pair = h // 2
                    qoff = (h % 2) * D
                    drain_feed_through(pair)
                    for qb in range(TB512):
                        npairs = 2 * qb + 2  # fused ki-pairs (4qb+4 k-tiles)
                        drain_v_through(2 * qb + 1)
                        av = ps_av.tile([D + 1, 512], f32, tag="av")
                        pend = None

                        def do_av(p, p_sb, av=av, h=h, npairs=npairs):
                            for half in range(2):
                                ki = 2 * p + half
                                nc.tensor.matmul(
                                    av,
                                    lhsT=vone_k[ki // 2][:, ki % 2, h, :],
                                    rhs=p_sb[:, _ts(half, 512)],
                                    start=(ki == 0),
                                    stop=(ki == 2 * npairs - 1),
                                )

                        for p in range(npairs):
                            sp = ps_sp.tile([128, 1024], f32, tag="sp")
                            for half in range(2):
                                ki = 2 * p + half
                                nc.tensor.matmul(
                                    sp[:, _ts(half, 512)],
                                    lhsT=kTs[pair][qoff : qoff + D, _ts(ki, 128)],
                                    rhs=qTs[pair][qoff : qoff + D, _ts(qb, 512)],
                                    start=True,
                                    stop=True,
                                )
                            if pend is not None:
                                do_av(*pend)
                            # keep the PE dense during ACT-paced stretches
                            step_no[0] += 1
                            pump(2)
                            p_sb = pp.tile([128, 1024], bf, tag="p")
                            nc.scalar.activation(p_sb, sp, Exp)
                            if p >= 2 * qb:  # both halves are diagonal tiles
                                j = 2 * (p - 2 * qb)
                                nc.vector.tensor_mul(
                                    p_sb,
                                    p_sb,
                                    msk_sb[:, j : j + 2, :].rearrange(
                                        "k j q -> k (j q)"
                                    ),
                                )
                            pend = (p, p_sb)
                            if p == 1:
                                while deferred:
                                    fn, dh, dqb = deferred.pop(0)
                                    fn()
                                    if dh == HPC - 1:
                                        cp_state["unlocked"] = 4 * (dqb + 1)
                            if pair == 3:
                                pump_cproj(2)
                        do_av(*pend)
                        deferred.append(
                            (
                                lambda av=av, pair=pair, qoff=qoff, qb=qb, idx=h
                                * TB512
                                + qb: normalize(av, pair, qoff, qb, idx),
                                h,
                                qb,
                            )
                        )
                while feed_state["alive"]:  # drain any remaining feed
                    pump(1)
                while deferred:
                    fn, dh, dqb = deferred.pop(0)
                    fn()
                cp_state["unlocked"] = TB128
                while next(cfeed, None) is not None:
                    pass


    nc.compile()
    return nc


def _part_major(a, p=128):
    """[n*128, m] -> [128, n, m] with partition index innermost in rows."""
    n = a.shape[0] // p
    return np.ascontiguousarray(a.reshape(n, p, a.shape[1]).transpose(1, 0, 2))


def make_in_maps(x, W_attn, b_attn, W_proj, with_bias=False):
    """Build the 8 per-core input maps (core = 2*b + g)."""
    x = np.asarray(x, dtype=np.float32)
    W_attn = np.asarray(W_attn, dtype=np.float32)
    b_attn = np.asarray(b_attn, dtype=np.float32)
    W_proj = np.asarray(W_proj, dtype=np.float32)

    # causal 0/1 masks for the 4 diagonal alignments (k-tile 128 vs q-block 512)
    kk = np.arange(128)[:, None]
    qq = np.arange(512)[None, :]
    msk = np.stack(
        [(qq >= j * 128 + kk) for j in range(4)], axis=1
    ).astype(BF16)  # [128, 4, 512]

    in_maps = []
    for b in range(B):
        xt = _part_major(np.ascontiguousarray(x[b].T)).astype(BF16)  # [128,8,S]
        for g in range(2):
            qs = W_attn[:, g * FPC : (g + 1) * FPC]
            ks = W_attn[:, E + g * FPC : E + (g + 1) * FPC]
            vs = W_attn[:, 2 * E + g * FPC : 2 * E + (g + 1) * FPC]
            wqk = _part_major(np.concatenate([qs, ks], axis=1)).astype(BF16)
            wv = _part_major(vs).astype(BF16)
            wp = _part_major(W_proj[g * FPC : (g + 1) * FPC, :]).astype(BF16)
            bq = b_attn[g * FPC : (g + 1) * FPC]
            bk = b_attn[E + g * FPC : E + (g + 1) * FPC]
            bqk = np.concatenate([bq, bk])[None, :].astype(BF16)
            bv = b_attn[2 * E + g * FPC : 2 * E + (g + 1) * FPC][None, :].astype(
                BF16
            )
            m = {
                "xt": xt,
                "wqk": np.ascontiguousarray(wqk),
                "wv": np.ascontiguousarray(wv),
                "wp": np.ascontiguousarray(wp),
                "msk": np.ascontiguousarray(msk),
            }
            if with_bias:
                m["bqk"] = np.ascontiguousarray(bqk)
                m["bv"] = np.ascontiguousarray(bv)
            in_maps.append(m)
    return in_maps


def get_program(with_bias=False):
    key = f"nc{int(with_bias)}"
    if key not in _cache:
        _cache[key] = _build_program(with_bias)
    return _cache[key]


def gather(results, b_proj):
    b_proj = np.asarray(b_proj, dtype=np.float32)
    out = np.empty((B, S, E), dtype=np.float32)
    for b in range(B):
        out[b] = results[2 * b]["out"] + results[2 * b + 1]["out"] + b_proj
    return out


def kernel(x, W_attn, b_attn, W_proj, b_proj):
    with_bias = bool(np.any(np.asarray(b_attn)))
    nc = get_program(with_bias)
    in_maps = make_in_maps(x, W_attn, b_attn, W_proj, with_bias=with_bias)
    res = bass_utils.run_bass_kernel_spmd(nc, in_maps, core_ids=list(range(NCORE)))
    return gather(res.results, b_proj)


# revision 20
# speedup vs baseline: 1.2512x; 1.2512x over previous
"""Trainium2 Bass kernel for causal multi-head attention (B=4, S=2048, E=1024, H=16).

Sharding: 8 cores = (batch b in 0..3) x (head-group g in 0..1); each core
computes one batch and 8 heads end-to-end:
  - column-parallel QKV projection (only its heads' columns)
  - causal attention for its 8 heads
  - row-parallel output projection -> partial [S, E]
The two partials per batch are summed on the host (plus b_proj). No on-device
collectives are needed.

Device dataflow (per core), all matmuls in bf16 with fp32 PSUM accumulation:
  - Q^T, K^T computed directly in [feature, token] layout (out = W^T @ x^T),
    so attention needs no transposes. K^T is pre-scaled by 1/sqrt(d)=0.125.
  - scores S^T[k,q] = (K^T tile).T @ Q^T, two k-tiles fused per [128,1024]
    PSUM tile; one ACT exp evacuates both; causal mask via multiplicative
    0/1 bf16 masks on diagonal tiles (fully-masked tiles skipped).
  - AV: lhsT = [V | ones]; PSUM rows 0..63 = O^T (unnormalized), row 64 =
    softmax denominator. 1/denom via ACT exp(-ln(d)) (DVE reciprocal is
    ~3.3us for a 1-partition row; ACT is ~0.9us), PE ones-matmul broadcast,
    one DVE multiply -> normalized O^T bf16. Normalize chains are deferred
    into the next (head, q-block) group so the PE never stalls on them.
  - All PSUM evacuation goes through ACT: DVE reads of PE-written PSUM race
    the matmul drain on HW (observed flaky garbage on first execution).
  - c_proj contracts the stacked O^T [512, S] against W_proj rows.
"""

import os
import sys

import numpy as np

for _p in ("/opt/trn_rl_repo", "/root/.axon_site/_ro/trn_rl_repo"):
    if os.path.isdir(_p) and _p not in sys.path:
        sys.path.append(_p)

import ml_dtypes  # noqa: E402

import concourse.bass as bass  # noqa: E402
import concourse.tile as tile  # noqa: E402
from concourse import bacc, bass_utils, hw_specs, mybir  # noqa: E402

# The act-table chooser assigns each activation the first table set containing
# its function, which ping-pongs Exp ("exp_and_others") and Ln ("natural_log")
# and inserts a 1.3us ACT_TABLE_LOAD per normalize chain (~65 per program).
# Restrict Exp/Ln to the combined "natural_log_exp_and_others" set (its
# act_func_set_id is preserved) so one load covers the whole kernel.
_orig_gat = hw_specs.get_activation_tables


def _shaped_gat(arch):
    t = _orig_gat(arch)
    if "natural_log_exp_and_others" in t:
        for name, funcs in t.items():
            if name != "natural_log_exp_and_others":
                funcs.discard(mybir.ActivationFunctionType.Exp)
                funcs.discard(mybir.ActivationFunctionType.Ln)
    return t


hw_specs.get_activation_tables = _shaped_gat
bacc.get_activation_tables = _shaped_gat

BF16 = ml_dtypes.bfloat16

B, S, E, H = 4, 2048, 1024, 16
D = E // H            # 64
NCORE = 8
HPC = H // 2          # heads per core = 8
KT = E // 128         # contraction tiles over E = 8
TB512 = S // 512      # 4
TB128 = S // 128      # 16
FPC = HPC * D         # features per core in attention output = 512

_cache: dict = {}


def _ts(i, n):
    return slice(i * n, (i + 1) * n)


def _build_program(with_bias):
    bf = mybir.dt.bfloat16
    f32 = mybir.dt.float32
    nc = bacc.Bacc("TRN2", target_bir_lowering=False, debug=False)

    xt = nc.dram_tensor("xt", [128, KT, S], bf, kind="ExternalInput")
    wqk = nc.dram_tensor("wqk", [128, KT, 2 * FPC], bf, kind="ExternalInput")
    wv = nc.dram_tensor("wv", [128, KT, FPC], bf, kind="ExternalInput")
    wp = nc.dram_tensor("wp", [128, FPC // 128, E], bf, kind="ExternalInput")
    if with_bias:
        bqk = nc.dram_tensor("bqk", [1, 2 * FPC], bf, kind="ExternalInput")
        bv = nc.dram_tensor("bv", [1, FPC], bf, kind="ExternalInput")
    msk = nc.dram_tensor("msk", [128, 4, 512], bf, kind="ExternalInput")
    out = nc.dram_tensor("out", [S, E], f32, kind="ExternalOutput")
    # scratch for the softmax-reciprocal partition broadcast (DMA bounce)
    rscr = nc.dram_tensor("rscr", [HPC * TB512, 512], mybir.dt.float32)

    Exp = mybir.ActivationFunctionType.Exp
    Ln = mybir.ActivationFunctionType.Ln

    with tile.TileContext(nc) as tc:
        with (
            tc.tile_pool(name="big", bufs=1) as big,
            tc.tile_pool(name="pp", bufs=3) as pp,
            tc.tile_pool(name="sm", bufs=3) as sm,
            tc.tile_pool(name="ob", bufs=3) as ob,
        ):
            xts = [
                big.tile([128, S], bf, tag=f"xt{k}", name=f"xt{k}")
                for k in range(KT)
            ]
            wqks = [
                big.tile([128, 2 * FPC], bf, tag=f"wqk{k}", name=f"wqk{k}")
                for k in range(KT)
            ]
            wv_sb = big.tile([128, KT, FPC], bf, tag="wv")
            wp_sb = big.tile([128, FPC // 128, E], bf, tag="wp")
            if with_bias:
                bqk_sb = big.tile([1, 2 * FPC], bf, tag="bqk")
                bv_sb = big.tile([1, FPC], bf, tag="bv")
            msk_sb = big.tile([128, 4, 512], bf, tag="msk")
            ones_sb = big.tile([1, S], bf, tag="ones")
            # per-pair Q^T/K^T/O^T tiles so interleaved QKV writes don't
            # WAR-serialize against another pair's attention reads
            qTs = [big.tile([128, S], bf, tag=f"qT{p}", name=f"qT{p}") for p in range(4)]
            kTs = [big.tile([128, S], bf, tag=f"kT{p}", name=f"kT{p}") for p in range(4)]
            oTs = [big.tile([128, S], bf, tag=f"oT{p}", name=f"oT{p}") for p in range(4)]
            vone_k = [
                big.tile([128, 2, HPC, D + 1], bf, tag=f"vone{tp}", name=f"vone{tp}")
                for tp in range(TB128 // 2)
            ]

            if with_bias:
                nc.sync.dma_start(out=bqk_sb, in_=bqk.ap())
                nc.sync.dma_start(out=bv_sb, in_=bv.ap())
            for k in range(KT):
                nc.sync.dma_start(out=wqks[k], in_=wqk.ap()[:, k, :])
                nc.sync.dma_start(out=xts[k], in_=xt.ap()[:, k, :])
            nc.sync.dma_start(out=wv_sb, in_=wv.ap())
            nc.sync.dma_start(out=msk_sb, in_=msk.ap())
            nc.sync.dma_start(out=wp_sb, in_=wp.ap())

            nc.vector.memset(ones_sb, 1.0)
            for tp in range(TB128 // 2):
                nc.vector.memset(vone_k[tp][:, :, :, D : D + 1], 1.0)

            # Attention-phase PSUM pools are also used to double-buffer the
            # upfront QKV/V projection groups (same [128,1024] tile shape).
            with (
                tc.tile_pool(name="sp", bufs=2, space="PSUM") as ps_sp,
                tc.tile_pool(name="av", bufs=2, space="PSUM") as ps_av,
                tc.tile_pool(name="qk", bufs=1, space="PSUM") as ps_qk,
            ):

                def emit_qk_group(fb, tbp, pool, tag):
                    """One Q^T/K^T projection group: 18 matmuls + 1 cast.
                    Yields after each instruction so it can be interleaved."""
                    ps = pool.tile([128, 1024], f32, tag=tag, name=f"qkv_{fb}_{tbp}")
                    for half in range(2):
                        tb = 2 * tbp + half
                        hs = _ts(half, 512)
                        for kt in range(KT):
                            nc.tensor.matmul(
                                ps[:, hs],
                                lhsT=wqks[kt][:, _ts(fb, 128)],
                                rhs=xts[kt][:, _ts(tb, 512)],
                                start=(kt == 0),
                                stop=(kt == KT - 1 and not with_bias),
                            )
                            yield
                        if with_bias:
                            nc.tensor.matmul(
                                ps[:, hs],
                                lhsT=bqk_sb[0:1, _ts(fb, 128)],
                                rhs=ones_sb[0:1, _ts(tb, 512)],
                                start=False,
                                stop=True,
                            )
                            yield
                    if fb < 4:
                        nc.scalar.copy(qTs[fb][:, _ts(tbp, 1024)], ps)
                    else:
                        nc.scalar.mul(kTs[fb - 4][:, _ts(tbp, 1024)], ps, 0.125)
                    yield

                # ---- Phase 1a: pair-0 Q^T/K^T upfront (dense) ----
                pools = [ps_qk, ps_sp]
                gi = 0
                for fb in (0, 4):
                    for tbp in range(TB512 // 2):
                        pool = pools[gi % 2]
                        for _ in emit_qk_group(
                            fb, tbp, pool, "qkvi" if pool is ps_qk else "sp"
                        ):
                            pass
                        gi += 1

                # ---- Phase 1b: V = x @ Wv (+bias), [token, feature] ----
                def emit_v_group(tbp, pool, tag):
                    ps = pool.tile([128, 1024], f32, tag=tag, name=f"v_{tbp}")
                    for half in range(2):
                        tb = 2 * tbp + half
                        hs = _ts(half, 512)
                        for kt in range(KT):
                            nc.tensor.matmul(
                                ps[:, hs],
                                lhsT=xts[kt][:, _ts(tb, 128)],
                                rhs=wv_sb[:, kt, :],
                                start=(kt == 0),
                                stop=(kt == KT - 1 and not with_bias),
                            )
                            yield
                        if with_bias:
                            nc.tensor.matmul(
                                ps[:, hs],
                                lhsT=ones_sb[0:1, 0:128],
                                rhs=bv_sb[0:1, :],
                                start=False,
                                stop=True,
                            )
                            yield
                    nc.scalar.copy(
                        vone_k[tbp][:, :, :, 0:D],
                        ps[:, :].rearrange("p (t h d) -> p t h d", t=2, d=D),
                    )
                    yield

                # V upfront (dense PE work; interleaving it into the
                # ACT-saturated attention phase measured slower)
                for tbp in range(TB128 // 2):
                    pool = pools[tbp % 2]
                    for _ in emit_v_group(
                        tbp, pool, "qkvi" if pool is ps_qk else "sp"
                    ):
                        pass

                # ---- feed: V tile-pairs 2..7 then QKV pairs 1..3,
                # interleaved into attention to keep the PE dense ----
                def main_feed():
                    for pr in range(1, 4):
                        for fb in (pr, pr + 4):
                            for tbp in range(TB512 // 2):
                                yield from (
                                    ("qk", pr)
                                    for _ in emit_qk_group(fb, tbp, ps_qk, "qkvi")
                                )

                feed = main_feed()
                # cur_pr semantics: pairs < cur_pr are fully emitted
                feed_state = {"cur_v": 2, "cur_pr": 1, "alive": True}

                def pump(n):
                    for _ in range(n):
                        got = next(feed, None)
                        if got is None:
                            feed_state["alive"] = False
                            feed_state["cur_v"] = 99
                            feed_state["cur_pr"] = 5
                            return
                        kind, idx = got
                        if kind == "v":
                            feed_state["cur_v"] = idx
                        else:
                            feed_state["cur_v"] = 99
                            feed_state["cur_pr"] = idx

                def drain_feed_through(pr):
                    """Emit QKV work until every pair <= pr is complete.
                    Required before attention reads qTs[pr]/kTs[pr]: Tile only
                    sees RAW deps for writes emitted before the read."""
                    while feed_state["alive"] and feed_state["cur_pr"] <= pr:
                        pump(1)

                def drain_v_through(tp):
                    """Emit V work until vone tile-pairs <= tp are complete."""
                    while feed_state["alive"] and feed_state["cur_v"] <= tp:
                        pump(1)

                # ---- c_proj feed: interleaved into pair-3 attention as the
                # QKV feed runs dry there (its work must precede pair 3).
                # A tb group unlocks once every head has normalized its
                # q-block (pair 3 / h=7 is the last writer of oTs[3]).
                def cproj_group(tb):
                    ps = ps_qk.tile([128, 1024], f32, tag="qkvi", name=f"pj{tb}")
                    for eb in range(2):
                        for fg in range(FPC // 128):
                            nc.tensor.matmul(
                                ps[:, _ts(eb, 512)],
                                lhsT=oTs[fg][:, _ts(tb, 128)],
                                rhs=wp_sb[:, fg, _ts(eb, 512)],
                                start=(fg == 0),
                                stop=(fg == FPC // 128 - 1),
                            )
                            yield
                    o_sb = ob.tile([128, 1024], f32, tag="osb", name=f"ob{tb}")
                    nc.scalar.copy(o_sb, ps)
                    yield
                    nc.sync.dma_start(out=out.ap()[_ts(tb, 128), :], in_=o_sb)
                    yield

                def cproj_feed():
                    for tb in range(TB128):
                        while tb >= cp_state["unlocked"]:
                            yield False  # not allowed yet; no emission
                        yield from (True for _ in cproj_group(tb))

                cp_state = {"unlocked": 0}
                cfeed = cproj_feed()

                def pump_cproj(n):
                    for _ in range(n):
                        got = next(cfeed, None)
                        if got is None or got is False:
                            return

                # ---- Phase 2: causal attention per head ----
                deferred = []  # pending normalize chains (one per group)

                def normalize(av, pair, qoff, qb, idx):
                    t1 = sm.tile([1, 512], f32, tag="t1")
                    nc.scalar.activation(t1, av[D : D + 1, :], Ln)
                    r_sb = sm.tile([1, 512], f32, tag="r")
                    nc.scalar.activation(r_sb, t1, Exp, scale=-1.0)
                    # broadcast 1/denom across partitions via a DRAM bounce
                    nc.sync.dma_start(out=rscr.ap()[idx : idx + 1, :], in_=r_sb)
                    bc_sb = sm.tile([D, 512], f32, tag="bcsb")
                    rap = rscr.ap()[idx : idx + 1, :]
                    bcast = bass.AP(
                        tensor=rap.tensor,
                        offset=rap.offset,
                        ap=[[0, D]] + list(rap.ap)[1:],
                    )
                    nc.sync.dma_start(out=bc_sb, in_=bcast)
                    # av is read by DVE well after the AV matmuls drained
                    # (the chain is deferred a full group) so the direct
                    # PSUM read is safe here.
                    nc.vector.tensor_mul(
                        oTs[pair][qoff : qoff + D, _ts(qb, 512)], av[0:D, :], bc_sb
                    )

                step_no = [0]
                for h in range(HPC):
                    pair = h // 2
                    qoff = (h % 2) * D
                    drain_feed_through(pair)
                    for qb in range(TB512):
                        npairs = 2 * qb + 2  # fused ki-pairs (4qb+4 k-tiles)
                        drain_v_through(2 * qb + 1)
                        av = ps_av.tile([D + 1, 512], f32, tag="av")
                        pend = None

                        def do_av(p, p_sb, av=av, h=h, npairs=npairs):
                            for half in range(2):
                                ki = 2 * p + half
                                nc.tensor.matmul(
                                    av,
                                    lhsT=vone_k[ki // 2][:, ki % 2, h, :],
                                    rhs=p_sb[:, _ts(half, 512)],
                                    start=(ki == 0),
                                    stop=(ki == 2 * npairs - 1),
                                )

                        for p in range(npairs):
                            sp = ps_sp.tile([128, 1024], f32, tag="sp")
                            for half in range(2):
                                ki = 2 * p + half
                                nc.tensor.matmul(
                                    sp[:, _ts(half, 512)],
                                    lhsT=kTs[pair][qoff : qoff + D, _ts(ki, 128)],
                                    rhs=qTs[pair][qoff : qoff + D, _ts(qb, 512)],
                                    start=True,
                                    stop=True,
                                )
                            if pend is not None:
                                do_av(*pend)
                            # keep the PE dense during ACT-paced stretches
                            step_no[0] += 1
                            pump(2)
                            p_sb = pp.tile([128, 1024], bf, tag="p")
                            nc.scalar.activation(p_sb, sp, Exp)
                            if p >= 2 * qb:  # both halves are diagonal tiles
                                j = 2 * (p - 2 * qb)
                                nc.vector.tensor_mul(
                                    p_sb,
                                    p_sb,
                                    msk_sb[:, j : j + 2, :].rearrange(
                                        "k j q -> k (j q)"
                                    ),
                                )
                            pend = (p, p_sb)
                            if p == 1:
                                while deferred:
                                    fn, dh, dqb = deferred.pop(0)
                                    fn()
                                    if dh == HPC - 1:
                                        cp_state["unlocked"] = 4 * (dqb + 1)
                            if pair == 3:
                                pump_cproj(2)
                        do_av(*pend)
                        deferred.append(
                            (
                                lambda av=av, pair=pair, qoff=qoff, qb=qb, idx=h
                                * TB512
                                + qb: normalize(av, pair, qoff, qb, idx),
                                h,
                                qb,
                            )
                        )
                while feed_state["alive"]:  # drain any remaining feed
                    pump(1)
                while deferred:
                    fn, dh, dqb = deferred.pop(0)
                    fn()
                cp_state["unlocked"] = TB128
                while next(cfeed, None) is not None:
                    pass


    nc.compile()
    return nc


def _part_major(a, p=128):
    """[n*128, m] -> [128, n, m] with partition index innermost in rows."""
    n = a.shape[0] // p
    return np.ascontiguousarray(a.reshape(n, p, a.shape[1]).transpose(1, 0, 2))


def make_in_maps(x, W_attn, b_attn, W_proj, with_bias=False):
    """Build the 8 per-core input maps (core = 2*b + g)."""
    x = np.asarray(x, dtype=np.float32)
    W_attn = np.asarray(W_attn, dtype=np.float32)
    b_attn = np.asarray(b_attn, dtype=np.float32)
    W_proj = np.asarray(W_proj, dtype=np.float32)

    # causal 0/1 masks for the 4 diagonal alignments (k-tile 128 vs q-block 512)
    kk = np.arange(128)[:, None]
    qq = np.arange(512)[None, :]
    msk = np.stack(
        [(qq >= j * 128 + kk) for j in range(4)], axis=1
    ).astype(BF16)  # [128, 4, 512]

    in_maps = []
    for b in range(B):
        xt = _part_major(np.ascontiguousarray(x[b].T)).astype(BF16)  # [128,8,S]
        for g in range(2):
            qs = W_attn[:, g * FPC : (g + 1) * FPC]
            ks = W_attn[:, E + g * FPC : E + (g + 1) * FPC]
            vs = W_attn[:, 2 * E + g * FPC : 2 * E + (g + 1) * FPC]
            wqk = _part_major(np.concatenate([qs, ks], axis=1)).astype(BF16)
            wv = _part_major(vs).astype(BF16)
            wp = _part_major(W_proj[g * FPC : (g + 1) * FPC, :]).astype(BF16)
            bq = b_attn[g * FPC : (g + 1) * FPC]
            bk = b_attn[E + g * FPC : E + (g + 1) * FPC]
            bqk = np.concatenate([bq, bk])[None, :].astype(BF16)
            bv = b_attn[2 * E + g * FPC : 2 * E + (g + 1) * FPC][None, :].astype(
                BF16
            )
            m = {
                "xt": xt,
                "wqk": np.ascontiguousarray(wqk),
                "wv": np.ascontiguousarray(wv),
                "wp": np.ascontiguousarray(wp),
                "msk": np.ascontiguousarray(msk),
            }
            if with_bias:
                m["bqk"] = np.ascontiguousarray(bqk)
                m["bv"] = np.ascontiguousarray(bv)
            in_maps.append(m)
    return in_maps


def get_program(with_bias=False):
    key = f"nc{int(with_bias)}"
    if key not in _cache:
        _cache[key] = _build_program(with_bias)
    return _cache[key]


def gather(results, b_proj):
    b_proj = np.asarray(b_proj, dtype=np.float32)
    out = np.empty((B, S, E), dtype=np.float32)
    for b in range(B):
        out[b] = results[2 * b]["out"] + results[2 * b + 1]["out"] + b_proj
    return out


def kernel(x, W_attn, b_attn, W_proj, b_proj):
    with_bias = bool(np.any(np.asarray(b_attn)))
    nc = get_program(with_bias)
    in_maps = make_in_maps(x, W_attn, b_attn, W_proj, with_bias=with_bias)
    res = bass_utils.run_bass_kernel_spmd(nc, in_maps, core_ids=list(range(NCORE)))
    return gather(res.results, b_proj)
